# revision 39
# baseline (speedup 1.0000x reference)
"""Paged KV-cache decode attention with ALiBi (Baichuan-style), fused
QKV + attention + output projection, tensor-parallel over heads across
8 Trainium2 NeuronCores.

Final design (bf16, long-moving-dim matmuls, masked-stationary scores,
sequence-length-truncated K/V, ordered DMA rings; ~144-157us HW vs
556us baseline, rel err 4.3e-3):
  - All matmul operands bf16; PSUM accumulation fp32.
  - QKV: out[4,1920], stationary = xT chunk [128,4], moving = weight
    columns (512/psum bank); weights stream through SBUF double-buffered.
  - Scores row r = b*5+h of a shared [20,512] psum tile produced by a
    MASKED stationary [128,20] (zero except column r) so the PE can
    write every row despite the quadrant base restriction; 20
    accumulating matmuls per tile, then 20-lane bias add + Exp.
  - K cache: packed per nt-section, only rows with sl_b > nt*512, in
    one [128, n_live*512] bf16 image; 4 section DMAs let scores of
    section nt start as soon as its bytes land.
  - V cache: per-sequence [128(t%128), nch_b, 640(h,d)] truncated
    tiles, all resident; attn@V batches the 5 heads of a sequence via
    a [128,5] stationary of transposed probs (block-diag extraction).
  - ALiBi bias + sequence mask baked into a host [20,2048] tensor
    (slope_h*t, -1e30 past seq_len); -slope_h*pos_b enters as the Exp
    activation's per-partition bias.
  - softmax row sums via activation accum_out; normalization by per-
    sequence [5,1] reciprocal tiles folded into the psum->sbuf copy.
  - DMA: scalar (ACT) HWDGE ring carries smalls -> wcat pairs -> kt
    sections -> vt tiles in consumption order; sync (SP) ring carries
    wo slices + output store; gpsimd handles the two-line scatters.
"""

import math
import os
import sys
from contextlib import ExitStack

import numpy as np
import ml_dtypes

sys.path.insert(0, "/opt/trn_rl_repo")

BF16 = ml_dtypes.bfloat16
FP8 = ml_dtypes.float8_e4m3
WSCALE = 1.0

B = 4
E = 5120
H = 40
D = 128
BS = 16
NB = 512
MB = 128
S = MB * BS  # 2048
NCORES = 8
HPC = H // NCORES   # 5 heads per core
EPC = HPC * D       # 640
NKC = E // 128      # 40 contraction chunks
NQKV = 3 * EPC      # 1920 qkv output columns per core
R = HPC * B         # 20 (b,h) pairs per core
NEG = -1.0e30


def _alibi_slopes(num_heads):
    cp2 = 2 ** int(math.floor(math.log2(num_heads)))
    base = 2.0 ** (-(2.0 ** (-(math.log2(cp2) - 3))))
    slopes = base ** np.arange(1, cp2 + 1, dtype=np.float64)
    if cp2 != num_heads:
        extra_base = 2.0 ** (-(2.0 ** (-(math.log2(2 * cp2) - 3))))
        n_rem = min(cp2, num_heads - cp2)
        extra = extra_base ** np.arange(1, 1 + 2 * n_rem, 2, dtype=np.float64)
        slopes = np.concatenate([slopes, extra])
    return slopes.astype(np.float32)


def _kt_sections(sl):
    """live rows (r-order) per nt section and their packed offsets."""
    live = [[r for r in range(R) if sl[r // HPC] > nt * 512] for nt in range(4)]
    off = [0] * 4
    acc = 0
    for nt in range(4):
        off[nt] = acc
        acc += len(live[nt])
    return live, off, acc


_PROGRAM_CACHE = {}
LAST_RESULTS = None  # BassKernelResults of the most recent run (for test.py)


def _build_program(pos):
    import concourse.bacc as bacc
    import concourse.bass as bass
    import concourse.tile as tile
    from concourse import mybir

    f32 = mybir.dt.float32
    bf16 = mybir.dt.bfloat16
    nc = bacc.Bacc()
    sl = tuple(p + 1 for p in pos)
    nch = [(s + 127) // 128 for s in sl]
    live, off, nlive = _kt_sections(sl)

    hT = nc.declare_dram_parameter("hT", [128, NKC * B], bf16, isOutput=False)
    wcat = nc.declare_dram_parameter("wcat", [128, NKC, NQKV], bf16, isOutput=False)
    kt = nc.declare_dram_parameter("kt", [128, nlive * 512], bf16, isOutput=False)
    vt = [
        nc.declare_dram_parameter(f"vt{b}", [128, nch[b], EPC], bf16, isOutput=False)
        for b in range(B)
    ]
    wo = nc.declare_dram_parameter("wo", [128, 10, HPC, 512], bf16, isOutput=False)
    term1 = nc.declare_dram_parameter("term1", [R, S], f32, isOutput=False)
    term2 = nc.declare_dram_parameter("term2", [R, 1], f32, isOutput=False)
    ident = nc.declare_dram_parameter("ident", [20, 20], bf16, isOutput=False)
    identf = nc.declare_dram_parameter("identf", [20, 20], f32, isOutput=False)
    outp = nc.declare_dram_parameter("outp", [B, E], bf16, isOutput=True)

    with tile.TileContext(nc) as tc, ExitStack() as ctx:
        consts = ctx.enter_context(tc.tile_pool(name="consts", bufs=1))
        wpool = ctx.enter_context(tc.tile_pool(name="wpool", bufs=3))
        wopool = ctx.enter_context(tc.tile_pool(name="wopool", bufs=4))
        sfpool = ctx.enter_context(tc.tile_pool(name="sfpool", bufs=2))
        psum = ctx.enter_context(tc.tile_pool(name="psum", bufs=8, space="PSUM"))

        # ---- small constants first on the scalar (ACT) ring ----
        hT_sb = consts.tile([128, NKC * B], bf16)
        nc.scalar.dma_start(out=hT_sb[:], in_=hT[:])
        ident_sb = consts.tile([20, 20], bf16)
        nc.scalar.dma_start(out=ident_sb[:], in_=ident[:])
        identf_sb = consts.tile([20, 20], f32)
        nc.scalar.dma_start(out=identf_sb[:], in_=identf[:])
        term1_sb = consts.tile([R, S], f32)
        nc.scalar.dma_start(out=term1_sb[:], in_=term1[:])
        term2_sb = consts.tile([R, 1], f32)
        nc.scalar.dma_start(out=term2_sb[:], in_=term2[:])

        qkv_sb = consts.tile([B, NQKV], bf16)
        qTm_sb = consts.tile([128, 21 * R + 1], bf16)  # masked: col r*21 live
        kT_sb = consts.tile([128, R], bf16)            # col = r = b*5+h
        attn_sb = consts.tile([R, S], bf16)            # row r
        attnT_sb = consts.tile([128, 16 * R], bf16)    # col = c*20 + r
        sums_sb = consts.tile([R, 4], f32)
        sum2_sb = consts.tile([R, 2], f32)
        sumt_sb = consts.tile([R, 1], f32)
        recip_sb = consts.tile([R, 1], f32)
        recip_row = consts.tile([1, R], f32)
        recip_b = [consts.tile([HPC, 1], f32, name=f"recipb{b}") for b in range(B)]
        ao_sb = [consts.tile([HPC, EPC], bf16, name=f"ao{b}") for b in range(B)]
        aoT_sb = consts.tile([128, R], bf16)           # col = h*B + b
        out_sb = consts.tile([B, E], bf16)

        nc.vector.memset(qTm_sb[:], 0.0)

        # ---- fused QKV projection: qkv[4, 1920], 4-chunk weight groups ----
        qkv_ps = [
            psum.tile([B, min(512, NQKV - nt * 512)], f32, tag="ps", name=f"qkv_ps{nt}")
            for nt in range(4)
        ]
        GC = 2
        for g in range(NKC // GC):
            wt = wpool.tile([128, GC * NQKV], bf16, tag="w")
            nc.scalar.dma_start(out=wt[:], in_=wcat[:, GC * g:GC * (g + 1), :])
            for kl in range(GC):
                kc = GC * g + kl
                for nt in range(4):
                    w = min(512, NQKV - nt * 512)
                    nc.tensor.matmul(
                        qkv_ps[nt][:],
                        lhsT=hT_sb[:, kc * B:(kc + 1) * B],
                        rhs=wt[:, kl * NQKV + nt * 512: kl * NQKV + nt * 512 + w],
                        start=(kc == 0),
                        stop=(kc == NKC - 1),
                    )

        # K sections as separate tiles (per-section score deps), same ring
        kt_sb = []
        for nt in range(4):
            lo, hi = off[nt] * 512, (off[nt] + len(live[nt])) * 512
            ktile = consts.tile([128, (hi - lo)], bf16, name=f"kts{nt}")
            nc.scalar.dma_start(out=ktile[:], in_=kt[:, lo:hi])
            kt_sb.append(ktile)
        pass1_bs = [b for b in range(B) if nch[b] <= 12]
        pass2_bs = [b for b in range(B) if nch[b] > 12]
        vt_order = sorted(pass1_bs, key=lambda b: nch[b]) + pass2_bs
        vt_sb = [None] * B
        for b in vt_order:
            vtile = consts.tile([128, nch[b], EPC], bf16, name=f"vt{b}")
            nc.scalar.dma_start(out=vtile[:], in_=vt[b][:])
            vt_sb[b] = vtile
        # Gate the sync ring (wo weights) on the last K section so the
        # o_proj weights do not steal bandwidth from the critical path;
        # they flow in the window where SDMA would otherwise idle.
        nc.sync.dma_start(out=outp[0:1, 0:1], in_=vt_sb[vt_order[-1]][0:1, 0, 0:1])

        for nt in range(4):
            w = min(512, NQKV - nt * 512)
            nc.scalar.copy(qkv_sb[:, nt * 512: nt * 512 + w], qkv_ps[nt][:])

        # ---- transpose q into masked stationaries, k into kT ----
        kT_r = kT_sb[:].rearrange("p (b five) -> p b five", b=B, five=HPC)
        qTm_r = qTm_sb[:, :420].rearrange("p (b rest) -> p b rest", b=B, rest=105)
        for h in range(HPC):
            tq = psum.tile([128, B], bf16, tag="ps", name=f"tq_{h}")
            nc.tensor.transpose(
                tq[:], qkv_sb[:, h * 128:(h + 1) * 128], ident_sb[:B, :B]
            )
            nc.vector.tensor_copy(qTm_r[:, :, 21 * h], tq[:])
        for h in range(HPC):
            tk = psum.tile([128, B], bf16, tag="ps", name=f"tk_{h}")
            nc.tensor.transpose(
                tk[:], qkv_sb[:, EPC + h * 128: EPC + (h + 1) * 128],
                ident_sb[:B, :B],
            )
            nc.vector.tensor_copy(kT_r[:, :, h], tk[:])

        # ---- scatter new-token K column / V row ----
        for r in range(R):
            b = r // HPC
            ntp = pos[b] // 512
            col = live[ntp].index(r) * 512 + pos[b] % 512
            nc.vector.tensor_copy(
                kt_sb[ntp][:, col:col + 1], kT_sb[:, r:r + 1]
            )
        for b in range(B):
            nc.gpsimd.dma_start(
                out=vt_sb[b][pos[b] % 128: pos[b] % 128 + 1, pos[b] // 128, :],
                in_=qkv_sb[b:b + 1, 2 * EPC:3 * EPC],
            )

        # ---- scores + softmax on [20, 512] tiles; sections 0-2 first,
        # then chunk transposes + AV for sequences that need no section-3
        # chunks (their V tiles arrive first), then section 3 + the rest ----
        def do_scores(nt):
            lo = nt * 512
            sp = psum.tile([R, 512], f32, tag="ps", name=f"sp{nt}")
            for i, r in enumerate(live[nt]):
                nc.tensor.matmul(
                    sp[:],
                    lhsT=qTm_sb[:, r * R:(r + 1) * R],
                    rhs=kt_sb[nt][:, i * 512:(i + 1) * 512],
                    start=(i == 0),
                    stop=(i == len(live[nt]) - 1),
                )
            sf = sfpool.tile([R, 512], f32, tag="sf")
            if live[nt]:
                nc.vector.tensor_add(sf[:], sp[:], term1_sb[:, lo: lo + 512])
            else:
                nc.vector.memset(sf[:], NEG)
            nc.scalar.activation(
                attn_sb[:, lo: lo + 512],
                sf[:],
                func=mybir.ActivationFunctionType.Exp,
                bias=term2_sb[:],
                accum_out=sums_sb[:, nt:nt + 1],
            )

        def do_transposes(c0, c1):
            for c in range(c0, c1):
                ta = psum.tile([128, R], bf16, tag="ps", name=f"ta{c}")
                nc.tensor.transpose(
                    ta[:], attn_sb[:, c * 128:(c + 1) * 128], ident_sb[:]
                )
                nc.vector.tensor_copy(attnT_sb[:, c * R:(c + 1) * R], ta[:])

        ncmax = max(nch)
        ao_ps = {}

        def do_av(b):
            ao0 = psum.tile([HPC, 512], f32, tag="ps", name=f"ao0_{b}")
            ao1 = psum.tile([HPC, EPC - 512], f32, tag="ps", name=f"ao1_{b}")
            ao_ps[b] = (ao0, ao1)
            for c in range(nch[b]):
                lt = attnT_sb[:, c * R + b * HPC: c * R + (b + 1) * HPC]
                nc.tensor.matmul(
                    ao0[:], lhsT=lt, rhs=vt_sb[b][:, c, 0:512],
                    start=(c == 0), stop=(c == nch[b] - 1),
                )
                nc.tensor.matmul(
                    ao1[:], lhsT=lt, rhs=vt_sb[b][:, c, 512:EPC],
                    start=(c == 0), stop=(c == nch[b] - 1),
                )

        for nt in range(3):
            do_scores(nt)
        do_transposes(0, min(12, ncmax))
        do_scores(3)
        do_transposes(min(12, ncmax), ncmax)

        # ---- softmax denominators -> per-sequence [5,1] recip tiles ----
        nc.vector.tensor_add(sum2_sb[:, 0:1], sums_sb[:, 0:1], sums_sb[:, 1:2])
        nc.vector.tensor_add(sum2_sb[:, 1:2], sums_sb[:, 2:3], sums_sb[:, 3:4])
        nc.vector.tensor_add(sumt_sb[:], sum2_sb[:, 0:1], sum2_sb[:, 1:2])
        nc.vector.reciprocal(recip_sb[:], sumt_sb[:])
        rr = psum.tile([1, R], f32, tag="ps", name="rr")
        nc.tensor.transpose(rr[:], recip_sb[:], identf_sb[:])
        nc.vector.tensor_copy(recip_row[:], rr[:])
        for b in range(B):
            rb = psum.tile([HPC, 1], f32, tag="ps", name=f"rb{b}")
            nc.tensor.transpose(
                rb[:], recip_row[:, b * HPC:(b + 1) * HPC], identf_sb[:1, :1]
            )
            nc.vector.tensor_copy(recip_b[b][:], rb[:])

        for b in vt_order:
            if b in pass1_bs:
                do_av(b)
        for b in pass2_bs:
            do_av(b)
        for b in range(B):
            ao0, ao1 = ao_ps[b]
            nc.scalar.activation(
                ao_sb[b][:, 0:512], ao0[:],
                func=mybir.ActivationFunctionType.Copy, scale=recip_b[b][:],
            )
            nc.scalar.activation(
                ao_sb[b][:, 512:EPC], ao1[:],
                func=mybir.ActivationFunctionType.Copy, scale=recip_b[b][:],
            )

        # ---- transpose attn_out diag blocks -> aoT [128, 20] (col h*B+b) ----
        for b in range(B):
            for h in range(HPC):
                to = psum.tile([128, HPC], bf16, tag="ps", name=f"to{b}_{h}")
                nc.tensor.transpose(
                    to[:], ao_sb[b][:, h * 128:(h + 1) * 128], ident_sb[:HPC, :HPC]
                )
                nc.vector.tensor_copy(
                    aoT_sb[:, h * B + b: h * B + b + 1], to[:, h:h + 1]
                )

        # ---- output projection: out[4, 5120]; wo streams on sync ring ----
        for jg in range(10):
            wt = wopool.tile([128, HPC * 512], bf16, tag="wo", name=f"wo{jg}")
            nc.sync.dma_start(out=wt[:], in_=wo[:, jg])
            op = psum.tile([B, 512], f32, tag="ps", name=f"op{jg}")
            for hc in range(HPC):
                nc.tensor.matmul(
                    op[:],
                    lhsT=aoT_sb[:, hc * B:(hc + 1) * B],
                    rhs=wt[:, hc * 512:(hc + 1) * 512],
                    start=(hc == 0),
                    stop=(hc == HPC - 1),
                )
            nc.scalar.copy(out_sb[:, jg * 512:(jg + 1) * 512], op[:])

        nc.sync.dma_start(out=outp[:], in_=out_sb[:])

    nc.compile()
    return nc


def _bf16(x):
    return np.ascontiguousarray(x.astype(BF16))


def _fp8(x):
    return np.ascontiguousarray(x.astype(FP8))


def _prepare_core_inputs(core, hT_full, qkv_bf, o_bf, k_bf, v_bf, bt, sl, pos,
                         ident, identf, slopes_all):
    hs = slice(core * HPC, (core + 1) * HPC)
    es = slice(core * EPC, (core + 1) * EPC)
    live, off, nlive = _kt_sections(sl)
    nch = [(s + 127) // 128 for s in sl]

    # Wcat [128, 40, 1920]: Wcat[p, kc, j] = W[kc*128+p, j]; q pre-scaled.
    wcat = np.concatenate(
        [qkv_bf[0][:, es], qkv_bf[1][:, es], qkv_bf[2][:, es]], axis=1
    )
    wcat = np.ascontiguousarray(wcat.reshape(NKC, 128, NQKV).transpose(1, 0, 2))

    # K^T per sequence/head: ktb[b][d, h, t]
    kg = k_bf[:, hs]   # [NB, 5, 16, 128] bf16
    ktb = []
    for b in range(B):
        kk = kg[bt[b]].transpose(1, 0, 2, 3).reshape(HPC, S, D)
        ktb.append(kk.transpose(2, 0, 1))           # [d, h, t]
    # packed live-section image [128, nlive*512]
    kt = np.empty((128, nlive * 512), BF16)
    for nt in range(4):
        for i, r in enumerate(live[nt]):
            b, h = divmod(r, HPC)
            c0 = (off[nt] + i) * 512
            kt[:, c0:c0 + 512] = ktb[b][:, h, nt * 512:(nt + 1) * 512]

    # vt[b] [128(t%128), nch, 640(h,d)] truncated
    vg = v_bf[:, hs]
    vts = {}
    for b in range(B):
        vv = vg[bt[b]].transpose(0, 2, 1, 3).reshape(S, HPC, D)   # [t, h, d]
        vts[f"vt{b}"] = np.ascontiguousarray(
            vv.reshape(16, 128, HPC * D).transpose(1, 0, 2)[:, :nch[b], :]
        )

    # wo [128, 10, 5, 512]: contiguous per-jg slices
    wo = np.ascontiguousarray(
        o_bf[es, :].reshape(HPC, 128, 10, 512).transpose(1, 2, 0, 3))

    # alibi: term1[r, t] = slope_h * t (masked), term2[r] = -slope_h*pos_b
    slopes = slopes_all[core * HPC:(core + 1) * HPC]
    t_idx = np.arange(S, dtype=np.float32)
    term1 = np.empty((B, HPC, S), np.float32)
    term2 = np.empty((B, HPC, 1), np.float32)
    for b in range(B):
        term1[b] = slopes[:, None] * t_idx[None, :]
        term1[b, :, sl[b]:] = NEG
        term2[b, :, 0] = -slopes * np.float32(pos[b])

    return dict(hT=hT_full, wcat=wcat, kt=kt, wo=wo,
                term1=np.ascontiguousarray(term1.reshape(R, S)),
                term2=np.ascontiguousarray(term2.reshape(R, 1)),
                ident=ident, identf=identf, **vts)


def kernel(**inputs):
    global LAST_RESULTS
    hidden = np.asarray(inputs["hidden_states"], np.float32)
    qkv_w = np.asarray(inputs["qkv_weight"], np.float32)
    o_w = np.asarray(inputs["o_proj_weight"], np.float32)
    k_cache = np.asarray(inputs["k_cache"], np.float32)
    v_cache = np.asarray(inputs["v_cache"], np.float32)
    bt = np.asarray(inputs["block_tables"]).astype(np.int64)
    sl = np.asarray(inputs["sequence_lengths"]).astype(np.int64)

    pos = tuple(int(x) - 1 for x in sl)
    slt = tuple(int(x) for x in sl)

    # Shared host-side conversions (bf16 once, slice per core after).
    qkv_bf = [
        _bf16(qkv_w[0] * np.float32(D ** -0.5)),
        _bf16(qkv_w[1]),
        _bf16(qkv_w[2]),
    ]
    o_bf = _bf16(o_w)
    k_bf = _bf16(k_cache)
    v_bf = _bf16(v_cache)
    hT_full = _bf16(
        hidden.T.reshape(NKC, 128, B).transpose(1, 0, 2).reshape(128, NKC * B)
    )
    ident = np.eye(20, dtype=BF16)
    identf = np.eye(20, dtype=np.float32)
    slopes_all = _alibi_slopes(H)

    in_maps = [
        _prepare_core_inputs(c, hT_full, qkv_bf, o_bf, k_bf, v_bf, bt, slt, pos,
                             ident, identf, slopes_all)
        for c in range(NCORES)
    ]

    if pos not in _PROGRAM_CACHE:
        _PROGRAM_CACHE[pos] = _build_program(pos)
    nc = _PROGRAM_CACHE[pos]

    from concourse.bass_utils import run_bass_kernel_spmd

    res = run_bass_kernel_spmd(
        nc,
        in_maps,
        core_ids=list(range(NCORES)),
        trace=bool(os.environ.get("BASS_TRACE")),
    )
    LAST_RESULTS = res

    out = np.zeros((B, E), np.float64)
    for c in range(NCORES):
        out += np.asarray(res.results[c]["outp"]).astype(np.float64)
    return out.astype(np.float32)
